# revision 6
# baseline (speedup 1.0000x reference)
"""Trainium2 Bass kernel for nn_BimodalAttentionSet.

The reference computes, per sample b and mode i:
    result_i[b] = mean_{j != i} ( A[(j,i)][b] @ x_i[b] )
where A[(j,i)][b] is the identity matrix whenever x_i[b] or x_j[b] has any
nonzero element, and row-softmax(outer) otherwise.  The softmax branch only
triggers when BOTH rows are entirely zero — but then the matvec operand
x_i[b] is itself the zero vector, so the term is 0 = x_i[b] there too.
Every term therefore equals x_i[b] and result_i == x_i bit-for-bit for ANY
input ((x+x)/2 is exact in f32).  The kernel is pure data movement:
out = stack([x0, x1, x2], axis=0) — which matches target_regime=memory.

Sharding: pure data parallelism over the batch dim B=2048 across 8 cores
(256 rows each).  Host-side, each core's three modality shards are stacked
into one contiguous [3*256, 256] f32 buffer; on-device each core copies its
768 KiB DRAM->DRAM as two concurrent copies on the two HWDGE rings
(Sync/SP and Scalar/ACT).  The split is ~60/40 toward Sync — traces show
the ACT ring completes equal-size transfers ~10% slower.  The unused
engine-preamble RegisterMoves on the two DMA engines are stripped from the
BIR to shorten the pre-issue critical path.  Measured ~340 GB/s per core
vs the ~358 GB/s per-NC HBM limit (~95% of the memory roofline for the
streaming phase); the rest of the ~11.4 us exec time is fixed NEFF/runtime
protocol (start event, engine barrier chains, instruction loads).
"""

import numpy as np

M = 3
N_CORES = 8

# Defaults for the spec'd problem size (B=2048, D=256); kernel() derives the
# actual values from its inputs and rebuilds if they differ.
B = 2048
D = 256
BS = B // N_CORES   # batch rows per core
R = M * BS          # stacked rows per core

_CACHE = {}


def _build_bass(rows, cols):
    import concourse.bass as bass
    import concourse.mybir as mybir

    class LeanBass(bass.Bass):
        """Skip the post-const-init all-engine barrier: nothing in this
        kernel reads the canonical const APs, and the walrus start protocol
        already synchronizes the engines."""

        def __init__(self, *a, **k):
            self._in_init = True
            super().__init__(*a, **k)
            self._in_init = False

        def all_engine_barrier(self, *, sem_only: bool = False):
            if getattr(self, "_in_init", False):
                return
            return super().all_engine_barrier(sem_only=sem_only)

    nc = LeanBass()
    dt = mybir.dt.float32
    x = nc.dram_tensor("x", [rows, cols], dt, kind="ExternalInput")
    out = nc.dram_tensor("out", [rows, cols], dt, kind="ExternalOutput")
    s_sem = nc.alloc_semaphore("s_sem")
    a_sem = nc.alloc_semaphore("a_sem")
    h = (rows * 29) // 48  # ~60% on the (faster) Sync/SP ring
    nc.sync.dma_start(out=out[:h], in_=x[:h]).then_inc(s_sem, 16)
    nc.scalar.dma_start(out=out[h:], in_=x[h:]).then_inc(a_sem, 16)
    # Sync is the SOLE waiter for both DMAs: Scalar heads straight into the
    # walrus end protocol, whose engine chain starts at Scalar — so the chain
    # prefix (Scalar->GpSimd->Vector) pre-fires while data still streams, and
    # only Sync's mid-chain slot gates on DMA completion (~0.2-0.4 us saved
    # on the end-chain ripple).  The chain cannot complete before Sync's
    # waits, so NEFF completion still implies all bytes landed.
    nc.sync.wait_ge(s_sem, 16)
    nc.sync.wait_ge(a_sem, 16)

    # Strip the unused engine-preamble RegisterMoves on the two DMA engines
    # from the serialized BIR: they sit between the walrus start protocol and
    # the dma_start on each engine's critical path (~0.3-0.5 us), and nothing
    # in this kernel reads those registers (verified bit-exact on HW).
    import orjson

    orig = type(nc).to_json_bytes

    def to_json_bytes():
        m = orjson.loads(orig(nc))
        for f in m["functions"]:
            for b in f["blocks"]:
                b["instructions"] = [
                    i for i in b["instructions"]
                    if not (
                        i.get("engine") in ("SP", "Activation")
                        and i.get("opcode") == "RegisterMove"
                    )
                ]
        return orjson.dumps(m)

    nc.to_json_bytes = to_json_bytes
    return nc


def kernel(x0: np.ndarray, x1: np.ndarray, x2: np.ndarray) -> np.ndarray:
    xs = [np.ascontiguousarray(np.asarray(x, dtype=np.float32)) for x in (x0, x1, x2)]
    b, d = xs[0].shape
    for x in xs:
        assert x.shape == (b, d), (x.shape, (b, d))

    # out == stack(xs) exactly (see module docstring); the device performs
    # the memory-roofline copy, sharded over the batch across the 8 cores.
    if b % (2 * N_CORES) != 0:
        # Shape outside the supported sharding — pure host fallback
        # (mathematically identical; never hit for the spec'd inputs).
        return np.stack(xs, axis=0)

    from concourse.bass_utils import run_bass_kernel_spmd

    bs = b // N_CORES
    rows = M * bs
    key = (rows, d)
    nc = _CACHE.get(key)
    if nc is None:
        nc = _CACHE[key] = _build_bass(rows, d)

    in_maps = [
        {
            "x": np.ascontiguousarray(
                np.stack([x[c * bs:(c + 1) * bs] for x in xs], axis=0)
            ).reshape(rows, d)
        }
        for c in range(N_CORES)
    ]
    res = run_bass_kernel_spmd(nc, in_maps, core_ids=list(range(N_CORES)))

    out = np.empty((M, b, d), dtype=np.float32)
    for c in range(N_CORES):
        out[:, c * bs:(c + 1) * bs, :] = res.results[c]["out"].reshape(M, bs, d)
    return out


# revision 7
# speedup vs baseline: 1.0436x; 1.0436x over previous
"""Trainium2 Bass kernel for nn_BimodalAttentionSet.

The reference computes, per sample b and mode i:
    result_i[b] = mean_{j != i} ( A[(j,i)][b] @ x_i[b] )
where A[(j,i)][b] is the identity matrix whenever x_i[b] or x_j[b] has any
nonzero element, and row-softmax(outer) otherwise.  The softmax branch only
triggers when BOTH rows are entirely zero — but then the matvec operand
x_i[b] is itself the zero vector, so the term is 0 = x_i[b] there too.
Every term therefore equals x_i[b] and result_i == x_i bit-for-bit for ANY
input ((x+x)/2 is exact in f32).  The kernel is pure data movement:
out = stack([x0, x1, x2], axis=0) — which matches target_regime=memory.

Sharding: pure data parallelism over the batch dim B=2048 across 8 cores
(256 rows each).  Host-side, each core's three modality shards are stacked
into one contiguous [3*256, 256] f32 buffer; on-device each core copies its
768 KiB DRAM->DRAM as two concurrent copies on the two HWDGE rings
(Sync/SP and Scalar/ACT).  The split is ~60/40 toward Sync — traces show
the ACT ring completes equal-size transfers ~10% slower.  The unused
engine-preamble RegisterMoves on the two DMA engines are stripped from the
BIR to shorten the pre-issue critical path.  Measured ~340 GB/s per core
vs the ~358 GB/s per-NC HBM limit (~95% of the memory roofline for the
streaming phase); the rest of the ~11.4 us exec time is fixed NEFF/runtime
protocol (start event, engine barrier chains, instruction loads).
"""

import numpy as np

M = 3
N_CORES = 8

# Defaults for the spec'd problem size (B=2048, D=256); kernel() derives the
# actual values from its inputs and rebuilds if they differ.
B = 2048
D = 256
BS = B // N_CORES   # batch rows per core
R = M * BS          # stacked rows per core

_CACHE = {}


def _build_bass(rows, cols):
    import concourse.bass as bass
    import concourse.mybir as mybir

    class LeanBass(bass.Bass):
        """Skip the post-const-init all-engine barrier: nothing in this
        kernel reads the canonical const APs, and the walrus start protocol
        already synchronizes the engines."""

        def __init__(self, *a, **k):
            self._in_init = True
            super().__init__(*a, **k)
            self._in_init = False

        def all_engine_barrier(self, *, sem_only: bool = False):
            if getattr(self, "_in_init", False):
                return
            return super().all_engine_barrier(sem_only=sem_only)

    nc = LeanBass()
    dt = mybir.dt.float32
    x = nc.dram_tensor("x", [rows, cols], dt, kind="ExternalInput")
    out = nc.dram_tensor("out", [rows, cols], dt, kind="ExternalOutput")
    sem = nc.alloc_semaphore("dma_sem")
    h = (rows * 29) // 48  # ~60% on the (faster) Sync/SP ring
    nc.sync.dma_start(out=out[:h], in_=x[:h]).then_inc(sem, 16)
    nc.scalar.dma_start(out=out[h:], in_=x[h:]).then_inc(sem, 16)
    # Sync is the SOLE waiter for both DMAs (single merged sem): Scalar heads
    # straight into the walrus end protocol, whose engine chain starts at
    # Scalar — so the chain prefix (Scalar->GpSimd->Vector) pre-fires while
    # data still streams, and only Sync's mid-chain slot gates on DMA
    # completion (~0.2-0.4 us saved on the end-chain ripple).  The chain
    # cannot complete before Sync's wait, so NEFF completion still implies
    # all bytes landed.
    nc.sync.wait_ge(sem, 32)

    # Strip the unused engine-preamble RegisterMoves on the two DMA engines
    # from the serialized BIR: they sit between the walrus start protocol and
    # the dma_start on each engine's critical path (~0.3-0.5 us), and nothing
    # in this kernel reads those registers (verified bit-exact on HW).
    import orjson

    orig = type(nc).to_json_bytes

    def to_json_bytes():
        m = orjson.loads(orig(nc))
        for f in m["functions"]:
            for b in f["blocks"]:
                b["instructions"] = [
                    i for i in b["instructions"]
                    if not (
                        i.get("engine") in ("SP", "Activation")
                        and i.get("opcode") == "RegisterMove"
                    )
                ]
        return orjson.dumps(m)

    nc.to_json_bytes = to_json_bytes
    return nc


def kernel(x0: np.ndarray, x1: np.ndarray, x2: np.ndarray) -> np.ndarray:
    xs = [np.ascontiguousarray(np.asarray(x, dtype=np.float32)) for x in (x0, x1, x2)]
    b, d = xs[0].shape
    for x in xs:
        assert x.shape == (b, d), (x.shape, (b, d))

    # out == stack(xs) exactly (see module docstring); the device performs
    # the memory-roofline copy, sharded over the batch across the 8 cores.
    if b % (2 * N_CORES) != 0:
        # Shape outside the supported sharding — pure host fallback
        # (mathematically identical; never hit for the spec'd inputs).
        return np.stack(xs, axis=0)

    from concourse.bass_utils import run_bass_kernel_spmd

    bs = b // N_CORES
    rows = M * bs
    key = (rows, d)
    nc = _CACHE.get(key)
    if nc is None:
        nc = _CACHE[key] = _build_bass(rows, d)

    in_maps = [
        {
            "x": np.ascontiguousarray(
                np.stack([x[c * bs:(c + 1) * bs] for x in xs], axis=0)
            ).reshape(rows, d)
        }
        for c in range(N_CORES)
    ]
    res = run_bass_kernel_spmd(nc, in_maps, core_ids=list(range(N_CORES)))

    out = np.empty((M, b, d), dtype=np.float32)
    for c in range(N_CORES):
        out[:, c * bs:(c + 1) * bs, :] = res.results[c]["out"].reshape(M, bs, d)
    return out
